# revision 81
# baseline (speedup 1.0000x reference)
"""Trainium2 Bass kernel for nn_DecoderBlock (upsample + skip-fusion + LN + Mamba).

Self-contained: hardcodes all shapes. Shards batch B=32 across 8 NeuronCores
(4 batches/core). Device layout is channels-first ([feature partitions, time]);
the host pre-transposes activations and pre-composes weights:
  * ConvTranspose1d(k=4,s=2,p=1) + channel-concat + 1x1 fusion conv
    -> 5 plain 512x512 matmuls (even/odd time phases + skip term)
  * LayerNorm mean removed exactly by column-centering those matrices;
    gamma folded into in_proj; var via ones-matmul on TensorE
  * depthwise conv k=4 -> 4 diagonal-matrix matmuls accumulated in PSUM
  * selective scan: decay powers a^(s+1) via ScalarE exps (+ a few DVE
    square-products); recurrence via tensor_tensor_scan (DVE-only on HW);
    B/C mults mostly on GpSimd; state-readout over S plus the Dp skip term
    on TensorE (identity/diagonal-matmul PSUM accumulation)

Schedule: per-batch emission is software-pipelined — stage1(b+1)
(front+LN+in_proj+conv+xproj+broadcasts) is emitted in chunks interleaved
into stage2(b) (z, scan, out_proj), because engine queues execute in
emission-priority order. The s-loop is skewed (bt-mult leads its scan by
2 states, the C-mult readout lags by 2) so GpSimd's in-order queue never
blocks behind a product waiting on a DVE scan. Silu is applied as batched
in-place acts to bound exp<->silu activation-table reloads (1283ns each);
stage-separated PSUM pools keep batches independent across the 8 banks.
"""
import numpy as np
import ml_dtypes

BF16 = ml_dtypes.bfloat16

D = 512        # d_model
DI = 1024      # d_inner
S = 16         # d_state
DTR = 32       # dt_rank
BTOT = 32      # total batch
TL = 512       # low-res time
T = 1024       # full time
NCORES = 8
BL = BTOT // NCORES   # batches per core

# ---- tuning knobs ----
import os as _os
def _env(k, d):
    return int(_os.environ.get(k, d))
DECAY_BF16 = True
# decay powers (s+1 in 1..16) computed as direct ScalarE exps; the rest are
# DVE products of two earlier powers.
CHAIN5 = {6: (3, 3), 8: (4, 4), 12: (6, 6), 14: (7, 7), 16: (8, 8)}
CHAIN8 = {6: (3, 3), 7: (3, 4), 8: (4, 4), 12: (3, 9), 13: (4, 9),
          14: (5, 9), 15: (6, 9), 16: (7, 9)}
CHAIN3 = {6: (3, 3), 8: (4, 4), 14: (7, 7)}
CHAIN0 = {}
CHAIN = {0: CHAIN0, 3: CHAIN3, 5: CHAIN5, 8: CHAIN8}[_env("K_NCHAIN", 3)]
# fraction knobs: every Nth bt/prod mult goes to GpSimd (Pool)
MULT_POOL_MOD = _env("K_MULT_POOL_MOD", 6)   # i % MOD == 0 -> Pool
BT_DVE_MOD = _env("K_BT_DVE_MOD", 0)         # if >0: bt i%MOD==0 -> DVE, else Pool; prod all Pool
BTPOOL_BUFS = _env("K_BTPOOL", 3)
HPOOL_BUFS = _env("K_HPOOL", 3)
PRPOOL_BUFS = _env("K_PRPOOL", 2)
UPOOL_BUFS = _env("K_UPOOL", 2)
DTPOOL_BUFS = _env("K_DTPOOL", 2)
SCAN_DVE_MOD = _env("K_SCAN_DVE_MOD", 0)     # 0 = all on Pool; else i%MOD==0 -> DVE
PFRONT_BUFS = _env("K_PFRONT", 1)
Z_LATE = _env("K_Z_LATE", 1)                 # 1: z-half after xproj/reps
OUT_DMA_ACT = _env("K_OUT_DMA_ACT", 0)
APOOL_BUFS = _env("K_APOOL", 4)
NCHAIN = _env("K_NCHAIN", 3)                 # 5: squares only; 8: deep
DIV_POOL = _env("K_DIV_POOL", 0)             # 1: silu divides on Pool
CHAIN_POOL = _env("K_CHAIN_POOL", 0)         # 1: decay chains on Pool
FN_POOL = _env("K_FN_POOL", 1)               # 1: LN-scale mult on Pool
Y_MODE = _env("K_Y_MODE", 2)                 # 0: y-extract on Act; 1: split; 2: DVE stt
SILU_PRIO = _env("K_SILU_PRIO", 200)        # high_priority offset for silu batches
XT_DVE = _env("K_XT_DVE", 0)                 # in_proj psum->sbuf copies on DVE
FT_DVE = _env("K_FT_DVE", 1)                 # front ft copy on DVE, sq as TT
OUT_DVE = _env("K_OUT_DVE", 1)               # out_proj psum->sbuf copies on DVE
SILU_DIV = _env("K_SILU_DIV", 2)             # 1: silu via exp+divide; 0: AF.Silu
SBUF_BCAST = _env("K_SBUF_BCAST", 0)         # 1: broadcast from SBUF (no DRAM hop)
GATE_POOL = _env("K_GATE_POOL", 1)           # 1: gating mult on Pool

_BUILT = None  # cached nc


def _host_prep(inputs):
    f32 = np.float32
    x = np.asarray(inputs["x"], f32)
    skip = np.asarray(inputs["skip"], f32)
    up_w = np.asarray(inputs["up_w"], f32)
    up_b = np.asarray(inputs["up_b"], f32)
    fus_w = np.asarray(inputs["fus_w"], f32)
    fus_b = np.asarray(inputs["fus_b"], f32)
    ln_g = np.asarray(inputs["ln_g"], f32)
    ln_b = np.asarray(inputs["ln_b"], f32)
    in_w = np.asarray(inputs["in_w"], f32)
    conv_w = np.asarray(inputs["conv_w"], f32)
    conv_b = np.asarray(inputs["conv_b"], f32)
    xproj_w = np.asarray(inputs["xproj_w"], f32)
    dt_w = np.asarray(inputs["dt_w"], f32)
    dt_b = np.asarray(inputs["dt_b"], f32)
    A_log = np.asarray(inputs["A_log"], f32)
    Dp = np.asarray(inputs["Dp"], f32)
    out_w = np.asarray(inputs["out_w"], f32)

    wt = np.swapaxes(up_w[:, :, ::-1], 0, 1)          # (out,in,k)
    fw_x, fw_s = fus_w[:, :D], fus_w[:, D:]
    M_e0 = fw_x @ wt[:, :, 0]
    M_e1 = fw_x @ wt[:, :, 2]
    M_o0 = fw_x @ wt[:, :, 1]
    M_o1 = fw_x @ wt[:, :, 3]
    fb = fw_x @ up_b + fus_b

    def center(M):
        return M - M.mean(axis=0, keepdims=True)

    mats = [center(m) for m in (M_e0, M_e1, M_o0, M_o1, fw_s)]
    fbc = fb - fb.mean()

    in_w_g = in_w * ln_g[None, :]
    c0 = in_w @ ln_b                                   # (2DI,)
    A = -np.exp(A_log[0, :]).astype(np.float64)        # (S,) rows identical
    assert np.abs(A_log - A_log[0:1, :]).max() == 0.0

    # --- device weight arrays (sliced-contiguous layouts) ---
    WT = np.stack([m.T.reshape(4, 128, 4, 128) for m in mats])   # (5,ki,kp,od,m)
    w_front = WT.transpose(2, 3, 0, 1, 4).astype(BF16).copy()    # (128,od,5,ki,128)
    w_in = in_w_g.T.reshape(4, 128, 16, 128).transpose(1, 2, 0, 3).astype(BF16).copy()
    w4 = conv_w[:, 0, :]                                         # (DI,4)
    w_conv = np.zeros((128, 8, 4, 128), f32)
    for dtile in range(8):
        for k in range(4):
            np.fill_diagonal(w_conv[:, dtile, k, :], w4[dtile * 128:(dtile + 1) * 128, k])
    w_conv = w_conv.astype(BF16)
    w_xp = xproj_w.T.reshape(8, 128, 64).transpose(1, 0, 2).astype(BF16).copy()  # (128,8,64)
    w_dt = np.zeros((128, DI), f32)
    w_dt[:DTR, :] = dt_w.T
    w_dt = w_dt.astype(BF16)
    w_out = out_w.T.reshape(8, 128, 4, 128).transpose(1, 2, 0, 3).astype(BF16).copy()
    w_dp = np.zeros((128, 8, 128), f32)
    for dtile in range(8):
        np.fill_diagonal(w_dp[:, dtile, :], Dp[dtile * 128:(dtile + 1) * 128])
    w_dp = w_dp.astype(BF16)
    ident = np.eye(128, dtype=f32).astype(BF16)
    ones = np.ones((128, 1), f32).astype(BF16)

    # biases packed [128, ncols]: fbc(4) c0x(8) c0z(8) conv_b(8) dt_b(8) Dp(8)
    # eps(1) -conv_b(8) -c0z(8)
    bias = np.zeros((128, 61), f32)
    bias[:, 0:4] = fbc.reshape(4, 128).T
    bias[:, 4:12] = c0[:DI].reshape(8, 128).T
    bias[:, 12:20] = c0[DI:].reshape(8, 128).T
    bias[:, 20:28] = conv_b.reshape(8, 128).T
    bias[:, 28:36] = dt_b.reshape(8, 128).T
    bias[:, 36:44] = Dp.reshape(8, 128).T
    bias[:, 44] = 1e-5
    bias[:, 45:53] = -conv_b.reshape(8, 128).T
    bias[:, 53:61] = -c0[DI:].reshape(8, 128).T

    # activations per core
    xs_ = x.transpose(0, 2, 1)                         # (B, D, TL)
    xpad = np.zeros((BTOT, D, TL + 2), f32)
    xpad[:, :, 1:TL + 1] = xs_
    xpad = xpad.astype(BF16)
    skT = skip.transpose(0, 2, 1)                      # (B, D, T)
    sk_e = skT[:, :, 0::2].astype(BF16).copy()
    sk_o = skT[:, :, 1::2].astype(BF16).copy()

    per_core = []
    for c in range(NCORES):
        sl = slice(c * BL, (c + 1) * BL)
        per_core.append(dict(
            xpad=np.ascontiguousarray(xpad[sl]),
            sk_e=np.ascontiguousarray(sk_e[sl]),
            sk_o=np.ascontiguousarray(sk_o[sl]),
        ))
    weights = dict(w_front=w_front, w_in=w_in, w_conv=w_conv, w_xp=w_xp,
                   w_dt=w_dt, w_out=w_out, w_dp=w_dp, ident=ident, ones=ones,
                   bias=bias)
    return per_core, weights, A


def _build(A, **_unused):
    import concourse.mybir as mybir
    import concourse.tile as tile
    from concourse import bacc
    from contextlib import ExitStack

    f32 = mybir.dt.float32
    bf16 = mybir.dt.bfloat16
    OP = mybir.AluOpType
    AF = mybir.ActivationFunctionType

    nc = bacc.Bacc("TRN2", target_bir_lowering=False, debug=False,
                   num_devices=NCORES)
    d_xpad = nc.dram_tensor("xpad", [BL, D, TL + 2], bf16, kind="ExternalInput")
    d_sk_e = nc.dram_tensor("sk_e", [BL, D, TL], bf16, kind="ExternalInput")
    d_sk_o = nc.dram_tensor("sk_o", [BL, D, TL], bf16, kind="ExternalInput")
    d_wf = nc.dram_tensor("w_front", [128, 4, 5, 4, 128], bf16, kind="ExternalInput")
    d_win = nc.dram_tensor("w_in", [128, 16, 4, 128], bf16, kind="ExternalInput")
    d_wcv = nc.dram_tensor("w_conv", [128, 8, 4, 128], bf16, kind="ExternalInput")
    d_wxp = nc.dram_tensor("w_xp", [128, 8, 64], bf16, kind="ExternalInput")
    d_wdt = nc.dram_tensor("w_dt", [128, DI], bf16, kind="ExternalInput")
    d_wout = nc.dram_tensor("w_out", [128, 4, 8, 128], bf16, kind="ExternalInput")
    d_id = nc.dram_tensor("ident", [128, 128], bf16, kind="ExternalInput")
    d_wdp = nc.dram_tensor("w_dp", [128, 8, 128], bf16, kind="ExternalInput")
    d_ones = nc.dram_tensor("ones", [128, 1], bf16, kind="ExternalInput")
    d_bias = nc.dram_tensor("bias", [128, 61], f32, kind="ExternalInput")
    d_out = nc.dram_tensor("outT", [BL, D, T], f32, kind="ExternalOutput")

    DEC_DT = bf16 if DECAY_BF16 else f32

    with tile.TileContext(nc) as tc:
        with ExitStack() as es:
            def pool(name, bufs, space="SBUF"):
                return es.enter_context(tc.tile_pool(name=name, bufs=bufs, space=space))
            cpool = pool("const", 1)
            wfpool = pool("wf", 1)
            wopool = pool("wout", 1)
            wpool = pool("wstream", 2)
            ipool = pool("inp", 4)
            fpool = pool("fused", 4)
            sqpool = pool("sq", 1)
            fnpool = pool("fn", 4)
            xinpool = pool("xin", 2)
            xdpool = pool("xdbl", 2)
            reppool = pool("rep", 1)      # 32 tags x 1 buf (8MB)
            rrpool = pool("rrep", 1)
            xspool = pool("xs", 8)        # xss resident per b
            dtpool = pool("dt", DTPOOL_BUFS)
            zpool = pool("z", 1)
            upool = pool("u", UPOOL_BUFS)
            apool = pool("a", APOOL_BUFS)
            btpool = pool("bt", BTPOOL_BUFS)
            hpool = pool("h", HPOOL_BUFS)
            prpool = pool("prod", PRPOOL_BUFS)
            ypool = pool("ysb", 8)
            spool = pool("small", 1)
            stile = pool("sil", 2)
            opool = pool("outs", 1)
            dpool = pool("dram", 3, "DRAM")
            # PSUM bank budget (8): front 1, mid 1, dt 1, out 1, readout 2, var 2
            pfront = pool("pfront", PFRONT_BUFS, "PSUM")
            pmid = pool("pmid", 1, "PSUM")
            pmd = pool("pmd", 1, "PSUM")
            pout = pool("pout", 1, "PSUM")
            pym = pool("py", 1, "PSUM")
            psm = pool("ps", 1, "PSUM")

            # ---------- small constants ----------
            wxp = cpool.tile([128, 8, 64], bf16)
            nc.sync.dma_start(wxp[:], d_wxp[:])
            wdt = cpool.tile([128, DI], bf16)
            nc.sync.dma_start(wdt[:], d_wdt[:])
            idt = cpool.tile([128, 128], bf16)
            nc.sync.dma_start(idt[:], d_id[:])
            wdp = cpool.tile([128, 8, 128], bf16)
            nc.sync.dma_start(wdp[:], d_wdp[:])
            onesb = cpool.tile([128, 1], bf16)
            nc.sync.dma_start(onesb[:], d_ones[:])
            bias = cpool.tile([128, 61], f32)
            nc.sync.dma_start(bias[:], d_bias[:])

            def bias_col(c):
                return bias[:, c:c + 1]

            mult_ctr = [0]

            def tt_engine():
                """Round-robin bt/prod mult engine: mostly Pool (scans own DVE),
                every MODth op on DVE."""
                i = mult_ctr[0]
                mult_ctr[0] += 1
                if MULT_POOL_MOD and i % MULT_POOL_MOD == 0:
                    return nc.vector
                return nc.gpsimd

            def scan_engine():
                return nc.vector

            def stage1(b, ctx):
                """Front + mid stage for batch b; yields at chunk boundaries
                so the driver can interleave its emission (and thus its
                engine-queue priority) into the previous batch's scan."""
                # ---------- load inputs ----------
                xp = []
                for ki in range(4):
                    t_ = ipool.tile([128, TL + 2], bf16, tag="xp", name="xp")
                    nc.sync.dma_start(t_[:], d_xpad[b, ki * 128:(ki + 1) * 128, :])
                    xp.append(t_)
                ske, sko = [], []
                for ki in range(4):
                    te = ipool.tile([128, TL], bf16, tag="ske", name="ske")
                    nc.sync.dma_start(te[:], d_sk_e[b, ki * 128:(ki + 1) * 128, :])
                    ske.append(te)
                    to = ipool.tile([128, TL], bf16, tag="sko", name="sko")
                    nc.sync.dma_start(to[:], d_sk_o[b, ki * 128:(ki + 1) * 128, :])
                    sko.append(to)

                # ---------- front end ----------
                fused = []   # per od: [128, 1024] bf16, blocked [even|odd]
                pss = psm.tile([1, 1024], f32, tag="pss", name="pss")
                for od in range(4):
                    wf = wfpool.tile([128, 5, 4, 128], bf16, tag="wf", name="wf")
                    nc.sync.dma_start(wf[:], d_wf[:, od])
                    pe = pfront.tile([128, 512], f32, tag="pmfr", name="pe")
                    po = pfront.tile([128, 512], f32, tag="pmfr", name="po")
                    n = 0
                    for ki in range(4):
                        nc.tensor.matmul(pe[:], wf[:, 0, ki, :], xp[ki][:, 0:TL],
                                         start=(n == 0), stop=False); n += 1
                        nc.tensor.matmul(pe[:], wf[:, 1, ki, :], xp[ki][:, 1:TL + 1],
                                         start=False, stop=False); n += 1
                        nc.tensor.matmul(pe[:], wf[:, 4, ki, :], ske[ki][:],
                                         start=False, stop=(n == 11)); n += 1
                    n = 0
                    for ki in range(4):
                        nc.tensor.matmul(po[:], wf[:, 2, ki, :], xp[ki][:, 1:TL + 1],
                                         start=(n == 0), stop=False); n += 1
                        nc.tensor.matmul(po[:], wf[:, 3, ki, :], xp[ki][:, 2:TL + 2],
                                         start=False, stop=False); n += 1
                        nc.tensor.matmul(po[:], wf[:, 4, ki, :], sko[ki][:],
                                         start=False, stop=(n == 11)); n += 1
                    ft = fpool.tile([128, 1024], bf16, tag="fused", name="ft")
                    sq = sqpool.tile([128, 1024], bf16, tag="sq", name="sq")
                    for ch, psrc in ((0, pe), (1, po)):
                        fh = ft[:, ch * 512:(ch + 1) * 512]
                        sh = sq[:, ch * 512:(ch + 1) * 512]
                        if FT_DVE:
                            nc.vector.tensor_scalar(out=fh, in0=psrc[:],
                                                    scalar1=bias_col(od),
                                                    scalar2=None, op0=OP.add)
                            nc.gpsimd.tensor_tensor(out=sh, in0=fh, in1=fh,
                                                    op=OP.mult)
                        else:
                            nc.scalar.activation(fh, psrc[:], AF.Identity,
                                                 bias=bias_col(od))
                            nc.scalar.activation(sh, psrc[:], AF.Square,
                                                 bias=bias_col(od))
                        nc.tensor.matmul(pss[:, ch * 512:(ch + 1) * 512], onesb[:],
                                         sh, start=(od == 0), stop=(od == 3))
                    fused.append(ft)
                    if od == 1:
                        yield

                # ---------- rstd ----------
                rst = spool.tile([1, T], bf16, tag="rstd", name="rst")
                for ch in range(2):
                    lnt = spool.tile([1, 512], bf16, tag="lnt", name="lnt")
                    nc.scalar.activation(lnt[:], pss[:, ch * 512:(ch + 1) * 512],
                                         AF.Ln, bias=bias[0:1, 44:45], scale=1.0 / D)
                    nc.scalar.activation(rst[:, ch * 512:(ch + 1) * 512], lnt[:],
                                         AF.Exp, scale=-0.5)
                rrep = rrpool.tile([128, T], bf16, tag="rrep", name="rrep")
                if SBUF_BCAST:
                    nc.sync.dma_start(rrep[:], rst[:].to_broadcast((128, T)))
                else:
                    d_rstd = dpool.tile([1, T], bf16, tag="drstd", name="d_rstd")
                    nc.sync.dma_start(d_rstd[:], rst[:])
                    nc.sync.dma_start(rrep[:], d_rstd[:].to_broadcast((128, T)))

                # ---------- LN scale (blocked in -> natural-t out) ----------
                fn = []
                for od in range(4):
                    t_ = fnpool.tile([128, T], bf16, tag="fn", name="fn")
                    (nc.gpsimd if FN_POOL else nc.vector).tensor_tensor(
                        out=t_[:].rearrange("p (a b) -> p b a", b=2),
                        in0=fused[od][:], in1=rrep[:], op=OP.mult)
                    fn.append(t_)
                ctx["fn"] = fn
                yield

                # ---------- in_proj (x half) + conv + silu ----------
                xss = []
                for mt in range(8):
                    wi = wpool.tile([128, 4, 128], bf16, tag="wi", name="wi")
                    nc.sync.dma_start(wi[:], d_win[:, mt])
                    xt = xinpool.tile([128, T + 3], bf16, tag="xin", name="xt")
                    nc.vector.memset(xt[:, 0:3], 0.0)
                    for ch in range(2):
                        pm = pmid.tile([128, 512], f32, tag="pm", name="pm")
                        for ki in range(4):
                            nc.tensor.matmul(pm[:], wi[:, ki, :],
                                             fn[ki][:, ch * 512:(ch + 1) * 512],
                                             start=(ki == 0), stop=(ki == 3))
                        if XT_DVE:
                            nc.vector.tensor_scalar(
                                out=xt[:, 3 + ch * 512:3 + (ch + 1) * 512],
                                in0=pm[:], scalar1=bias_col(4 + mt),
                                scalar2=None, op0=OP.add)
                        else:
                            nc.scalar.activation(
                                xt[:, 3 + ch * 512:3 + (ch + 1) * 512], pm[:],
                                AF.Identity, bias=bias_col(4 + mt))
                    # depthwise conv k=4 + silu
                    wc = wpool.tile([128, 4, 128], bf16, tag="wc", name="wc")
                    nc.sync.dma_start(wc[:], d_wcv[:, mt])
                    xst = xspool.tile([128, T], bf16, tag="xs", name="xst")
                    for ch in range(2):
                        pm = pmid.tile([128, 512], f32, tag="pm", name="pmc")
                        for k in range(4):
                            nc.tensor.matmul(pm[:], wc[:, k, :],
                                             xt[:, k + ch * 512:k + (ch + 1) * 512],
                                             start=(k == 0), stop=(k == 3))
                        if SILU_DIV:
                            # silu(v) = v/(1+exp(-v)), all on the exp table:
                            # e=exp(-v), v=Identity, den=e+1 [DVE 4x], TT div [2x]
                            et = stile.tile([128, 512], bf16, tag="sil", name="et")
                            nc.scalar.activation(et[:], pm[:], AF.Exp, scale=-1.0,
                                                 bias=bias_col(45 + mt))
                            xh = xst[:, ch * 512:(ch + 1) * 512]
                            nc.scalar.activation(xh, pm[:], AF.Identity,
                                                 bias=bias_col(20 + mt))
                            nc.vector.tensor_scalar(out=et[:], in0=et[:], scalar1=1.0,
                                                    scalar2=None, op0=OP.add)
                            nc.vector.tensor_tensor(out=xh, in0=xh,
                                                    in1=et[:], op=OP.divide)
                        elif SILU_DIV in (2, 3):
                            nc.scalar.activation(xst[:, ch * 512:(ch + 1) * 512],
                                                 pm[:], AF.Identity,
                                                 bias=bias_col(20 + mt))
                        else:
                            nc.scalar.activation(xst[:, ch * 512:(ch + 1) * 512],
                                                 pm[:], AF.Silu, bias=bias_col(20 + mt))
                    xss.append(xst)
                    if mt % 2 == 1:
                        yield
                if SILU_DIV in (2, 3):
                    # batched in-place silus: one contiguous act-table segment.
                    # high_priority sorts the batch below the co-running scan's
                    # decay exps so the scheduler can't shatter it (each entry/
                    # exit of the silu table costs a 1283ns table load).
                    with tc.high_priority(offset=SILU_PRIO):
                        for mt in range(8):
                            nc.scalar.activation(xss[mt][:], xss[mt][:], AF.Silu)
                ctx["xss"] = xss

                # ---------- xproj ----------
                xdb = xdpool.tile([64, T], bf16, tag="xdbl", name="xdb")
                for ch in range(2):
                    pm = pmid.tile([64, 512], f32, tag="pm", name="pmx")
                    for ki in range(8):
                        nc.tensor.matmul(pm[:], wxp[:, ki, :],
                                         xss[ki][:, ch * 512:(ch + 1) * 512],
                                         start=(ki == 0), stop=(ki == 7))
                    nc.scalar.activation(xdb[:, ch * 512:(ch + 1) * 512], pm[:], AF.Copy)
                if not SBUF_BCAST:
                    d_bc = dpool.tile([32, T], bf16, tag="dbc", name="d_bc")
                    nc.sync.dma_start(d_bc[:], xdb[32:64, :])
                ctx["xdb"] = xdb
                yield

                # ---------- B/C broadcasts (all 16 states) ----------
                brep, crep = {}, {}
                for s in range(S):
                    bt_ = reppool.tile([128, T], bf16, tag=f"brep{s}", name="brt")
                    bsrc = xdb[32 + s:33 + s, :] if SBUF_BCAST else d_bc[s:s + 1, :]
                    nc.sync.dma_start(bt_[:], bsrc.to_broadcast((128, T)))
                    brep[s] = bt_
                    ct_ = reppool.tile([128, T], bf16, tag=f"crep{s}", name="crt")
                    csrc = (xdb[48 + s:49 + s, :] if SBUF_BCAST
                            else d_bc[S + s:S + s + 1, :])
                    nc.sync.dma_start(ct_[:], csrc.to_broadcast((128, T)))
                    crep[s] = ct_
                    if s == 7:
                        yield
                ctx["brep"], ctx["crep"] = brep, crep

            def stage2(b, ctx, nxt):
                """z + scan + out for batch b, advancing the next batch's
                stage1 generator between chunks."""
                def adv():
                    if nxt is not None:
                        next(nxt, None)

                fn, xss, xdb = ctx["fn"], ctx["xss"], ctx["xdb"]
                brep, crep = ctx["brep"], ctx["crep"]

                # ---------- z half of in_proj ----------
                zbig = zpool.tile([128, 8, T], bf16, tag="z", name="zbig")
                zs = [zbig[:, zd, :] for zd in range(8)]
                for zd in range(8):
                    wi = wpool.tile([128, 4, 128], bf16, tag="wi", name="wiz")
                    nc.sync.dma_start(wi[:], d_win[:, 8 + zd])
                    zt = zs[zd]
                    for ch in range(2):
                        pm = pmid.tile([128, 512], f32, tag="pm", name="pmz")
                        for ki in range(4):
                            nc.tensor.matmul(pm[:], wi[:, ki, :],
                                             fn[ki][:, ch * 512:(ch + 1) * 512],
                                             start=(ki == 0), stop=(ki == 3))
                        if SILU_DIV:
                            et = stile.tile([128, 512], bf16, tag="sil", name="etz")
                            nc.scalar.activation(et[:], pm[:], AF.Exp, scale=-1.0,
                                                 bias=bias_col(53 + zd))
                            zh = zt[:, ch * 512:(ch + 1) * 512]
                            nc.scalar.activation(zh, pm[:], AF.Identity,
                                                 bias=bias_col(12 + zd))
                            nc.vector.tensor_scalar(out=et[:], in0=et[:], scalar1=1.0,
                                                    scalar2=None, op0=OP.add)
                            nc.vector.tensor_tensor(out=zh, in0=zh,
                                                    in1=et[:], op=OP.divide)
                        elif SILU_DIV == 2:
                            nc.scalar.activation(zt[:, ch * 512:(ch + 1) * 512],
                                                 pm[:], AF.Identity,
                                                 bias=bias_col(12 + zd))
                        elif SILU_DIV == 3:
                            nc.scalar.activation(zt[:, ch * 512:(ch + 1) * 512],
                                                 pm[:], AF.Silu, bias=bias_col(12 + zd))
                        else:
                            nc.scalar.activation(zt[:, ch * 512:(ch + 1) * 512],
                                                 pm[:], AF.Silu, bias=bias_col(12 + zd))
                if SILU_DIV == 2:
                    with tc.high_priority(offset=SILU_PRIO):
                        zflat = zbig[:].rearrange("p a b -> p (a b)")
                        nc.scalar.activation(zflat, zflat, AF.Silu)
                adv()

                # ---------- scan block: dtile outer, 16 states inner ----------
                ys = []
                for dt_ in range(8):
                    # dt = softplus(dt_w @ dtr + dt_b), lazily per dtile
                    msl = slice(dt_ * 128, (dt_ + 1) * 128)
                    dtt = dtpool.tile([128, T], bf16, tag="dt", name="dtt")
                    pe2 = spool.tile([128, T], bf16, tag="pe2", name="pe2")
                    for ch in range(2):
                        pm = pmd.tile([128, 512], f32, tag="pmd", name="pmd")
                        nc.tensor.matmul(pm[:], wdt[0:DTR, msl],
                                         xdb[0:DTR, ch * 512:(ch + 1) * 512],
                                         start=True, stop=True)
                        nc.scalar.activation(pe2[:, ch * 512:(ch + 1) * 512], pm[:],
                                             AF.Exp, bias=bias_col(28 + dt_))
                    nc.scalar.activation(dtt[:], pe2[:], AF.Ln, bias=1.0)
                    ut = upool.tile([128, T], bf16, tag="u", name="ut")
                    nc.gpsimd.tensor_tensor(out=ut[:], in0=dtt[:],
                                            in1=xss[dt_][:], op=OP.mult)
                    pys = [pym.tile([128, 512], f32, tag="py0", name="py0"),
                           pym.tile([128, 512], f32, tag="py1", name="py1")]
                    atiles = {}
                    bts = {}
                    hts = {}
                    # software-pipelined s-loop: bt leads the scan by 2, prod
                    # lags by 2, so Pool's in-order queue never blocks behind a
                    # prod that waits on a DVE scan.
                    for k in range(S + 2):
                        if k < S:
                            p = k + 1          # decay power = s+1
                            at = apool.tile([128, T], DEC_DT, tag="a", name="at")
                            if p in CHAIN:
                                p0, p1 = CHAIN[p]
                                (nc.gpsimd if CHAIN_POOL else nc.vector).tensor_tensor(
                                    out=at[:], in0=atiles[p0][:],
                                    in1=atiles[p1][:], op=OP.mult)
                            else:
                                nc.scalar.activation(at[:], dtt[:], AF.Exp,
                                                     scale=float(A[k]))
                            atiles[p] = at
                            btt = btpool.tile([128, T], bf16, tag="bt", name="btt")
                            if BT_DVE_MOD:
                                e = nc.vector if (k % BT_DVE_MOD == 0) else nc.gpsimd
                            else:
                                e = tt_engine()
                            e.tensor_tensor(out=btt[:], in0=ut[:],
                                            in1=brep[k][:], op=OP.mult)
                            bts[k] = btt
                        if 1 <= k <= S:
                            si = k - 1
                            ht = hpool.tile([128, T], bf16, tag="h", name="ht")
                            scan_engine().tensor_tensor_scan(
                                out=ht[:], data0=atiles[si + 1][:], data1=bts[si][:],
                                initial=0.0, op0=OP.mult, op1=OP.add)
                            hts[si] = ht
                        if k >= 2:
                            si = k - 2
                            pt = prpool.tile([128, T], bf16, tag="prod", name="pt")
                            if BT_DVE_MOD:
                                e = nc.vector if (si % 4 == 1) else nc.gpsimd
                            else:
                                e = tt_engine()
                            e.tensor_tensor(out=pt[:], in0=hts[si][:],
                                            in1=crep[si][:], op=OP.mult)
                            for ch in range(2):
                                csl = slice(ch * 512, (ch + 1) * 512)
                                nc.tensor.matmul(pys[ch][:], idt[:], pt[:, csl],
                                                 start=(si == 0), stop=False)
                    # Dp * xss folded into the PSUM accumulation (closes it)
                    for ch in range(2):
                        csl = slice(ch * 512, (ch + 1) * 512)
                        nc.tensor.matmul(pys[ch][:], wdp[:, dt_, :],
                                         xss[dt_][:, csl], start=False, stop=True)
                    yt = ypool.tile([128, T], bf16, tag="ysb", name="yt")
                    ys.append(yt)
                    for ch in range(2):
                        csl = slice(ch * 512, (ch + 1) * 512)
                        if Y_MODE == 2 or (Y_MODE == 1 and ch == 1):
                            nc.vector.tensor_scalar(
                                out=yt[:, csl], in0=pys[ch][:], scalar1=1.0,
                                scalar2=None, op0=OP.mult)
                        else:
                            nc.scalar.activation(yt[:, csl], pys[ch][:], AF.Identity)
                    # gating with pre-computed silu(z)
                    (nc.gpsimd if GATE_POOL else nc.vector).tensor_tensor(
                        out=yt[:], in0=yt[:], in1=zs[dt_], op=OP.mult)
                    adv()

                # ---------- out_proj ----------
                for od in range(4):
                    wo = wopool.tile([128, 8, 128], bf16, tag="wo", name="wo")
                    nc.sync.dma_start(wo[:], d_wout[:, od])
                    for ch in range(2):
                        pm = pout.tile([128, 512], f32, tag="pmo", name="pmo")
                        for ki in range(8):
                            nc.tensor.matmul(pm[:], wo[:, ki, :],
                                             ys[ki][:, ch * 512:(ch + 1) * 512],
                                             start=(ki == 0), stop=(ki == 7))
                        ot = opool.tile([128, 512], f32, tag="out", name="ot")
                        if OUT_DVE:
                            nc.vector.tensor_scalar(out=ot[:], in0=pm[:],
                                                    scalar1=0.0, scalar2=None,
                                                    op0=OP.add)
                        else:
                            nc.scalar.activation(ot[:], pm[:], AF.Copy)
                        (nc.scalar if OUT_DMA_ACT else nc.sync).dma_start(
                            d_out[b, od * 128:(od + 1) * 128, ch * 512:(ch + 1) * 512],
                            ot[:])
                if nxt is not None:
                    for _ in nxt:
                        pass

            ctxs = [dict() for _ in range(BL)]
            g = stage1(0, ctxs[0])
            for _ in g:
                pass
            for b in range(BL):
                nxt = stage1(b + 1, ctxs[b + 1]) if b + 1 < BL else None
                stage2(b, ctxs[b], nxt)
    nc.finalize()
    return nc


TRACE = False


def kernel(**inputs):
    global _BUILT
    per_core, weights, A = _host_prep(inputs)
    if _BUILT is None:
        _BUILT = _build(A)
    nc = _BUILT
    from concourse.bass_utils import run_bass_kernel_spmd
    in_maps = []
    for c in range(NCORES):
        m = dict(weights)
        m.update(per_core[c])
        in_maps.append(m)
    res = run_bass_kernel_spmd(nc, in_maps, core_ids=list(range(NCORES)),
                               trace=TRACE)
    globals()["LAST_RESULT"] = res
    out = np.empty((BTOT, T, D), np.float32)
    for c in range(NCORES):
        outT = res.results[c]["outT"]          # (BL, D, T)
        out[c * BL:(c + 1) * BL] = outT.transpose(0, 2, 1)
    return out
